# revision 14
# baseline (speedup 1.0000x reference)
"""Trainium2 Bass kernel for nn_DeepLatent loss (chamfer + L2 of a per-point MLP).

Strategy (8 cores, data-parallel over batch B=32 -> 4 samples/core):
  Per core, per sample s (channel-major layout: activations stored [C, Npoints]):
    h1 = relu(W1o.T @ obs^T + latbias)        latbias precomputed on host
    h2 = relu(W2.T @ h1 + b2)
    h3 = relu(W3.T @ h2 + b3)
    delta = W4.T @ h3                         est = obs + delta + b4
  Chamfer via a single augmented gram J = -d^2/2 (K=7 matmul):
    J[n,m] = gt_n . est_m - |est_m|^2/2 - |gt_n|^2/2
    lhsT rows: gt coords (3) | -0.5 (3) | -0.5*|gt|^2 (1, host-precomputed)
    rhs  rows: est coords (3) | est^2 (3) | ones (1)
    dir1 (per gt):  fused DVE tensor_tensor_reduce: J tile -> bf16 SBUF copy
                    + row-max accumulator in one pass.
    dir2 (per est): running elementwise bf16 max across the 8 gt tiles, then
                    8 PE transposes + one 3D-AP max-reduce.
  Sample s occupies partition band 32s (PE small-operand bases must be
  0/32/64/96 and lhsT/rhs must share a base).
  Scalar partials (max-sums, est^2 sum, cross sum) are combined on the host;
  sum(gt^2) is a pure input function computed on host.
"""

import ml_dtypes
import numpy as np
from contextlib import ExitStack

import concourse.bass as bass
import concourse.bacc as bacc
import concourse.mybir as mybir
import concourse.tile as tile
from concourse.bass_utils import run_bass_kernel_spmd

F32 = mybir.dt.float32
BF16 = mybir.dt.bfloat16
FP8 = mybir.dt.float8e4
AX = mybir.AxisListType
OP = mybir.AluOpType
ACTF = mybir.ActivationFunctionType

B, N, L = 32, 1024, 256
NCORES = 8
BS = B // NCORES  # samples per core
NT = N // 128     # gram tiles per sample
NEG = -3.0e38

# test.py hooks
TRACE = False
LAST = None
DEBUG_DUMPS = False


def build_program():
    nc = bacc.Bacc()

    obs_d = nc.dram_tensor("obs_t", [128, N], BF16, kind="ExternalInput")[:]
    ainit_d = nc.dram_tensor("a_init", [128, N], BF16, kind="ExternalInput")[:]
    cinit_d = nc.dram_tensor("c_init", [128, N], BF16, kind="ExternalInput")[:]
    obs2_d = nc.dram_tensor("obs_t2", [32, N], BF16, kind="ExternalInput")[:]
    ainit2_d = nc.dram_tensor("a_init2", [32, N], BF16, kind="ExternalInput")[:]
    cinit2_d = nc.dram_tensor("c_init2", [32, N], BF16, kind="ExternalInput")[:]
    latb_d = nc.dram_tensor("latb_t", [128, 4, BS], F32, kind="ExternalInput")[:]
    W1od = nc.dram_tensor("w1o4", [128, 512], BF16, kind="ExternalInput")[:]
    eye3d = nc.dram_tensor("eye34", [128, 3], BF16, kind="ExternalInput")[:]
    eye128d = nc.dram_tensor("eye128", [128, 128], BF16, kind="ExternalInput")[:]
    W2d = nc.dram_tensor("w2p", [128, 4, 512], FP8, kind="ExternalInput")[:]
    b2d = nc.dram_tensor("b2p", [128, 4], F32, kind="ExternalInput")[:]
    W3d = nc.dram_tensor("w3p", [128, 4, 256], FP8, kind="ExternalInput")[:]
    b3d = nc.dram_tensor("b3p", [128, 2], F32, kind="ExternalInput")[:]
    W4d = nc.dram_tensor("w4p", [128, 2, 3], BF16, kind="ExternalInput")[:]
    b4d = nc.dram_tensor("b4p", [3, 1], F32, kind="ExternalInput")[:]
    out_d = nc.dram_tensor("partials", [1, 8], F32, kind="ExternalOutput")[:]
    if DEBUG_DUMPS:
        m1_d = nc.dram_tensor("dbg_m1", [128, NT * BS], F32, kind="ExternalOutput")[:]
        m2_d = nc.dram_tensor("dbg_m2", [128, NT * BS], F32, kind="ExternalOutput")[:]
        r0_d = nc.dram_tensor("dbg_r0", [128, N], BF16, kind="ExternalOutput")[:]
        r1_d = nc.dram_tensor("dbg_r1", [128, N], BF16, kind="ExternalOutput")[:]

    with tile.TileContext(nc) as tc, ExitStack() as ctx:
        singles = ctx.enter_context(tc.tile_pool(name="singles", bufs=1))

        def fixed(shape, name, dtype=F32):
            return singles.tile(shape, dtype, tag=name, name=name)

        # ---------- fixed tiles ----------
        w1o4 = fixed([128, 512], "w1o4", BF16)
        eye34 = fixed([128, 3], "eye34", BF16)
        eye128 = fixed([128, 128], "eye128", BF16)
        latb = fixed([128, 4, BS], "latb")
        w2t = fixed([128, 4, 512], "w2t", FP8)
        w3t = fixed([128, 4, 256], "w3t", FP8)
        w4t = fixed([128, 2, 3], "w4t", BF16)
        b2t = fixed([128, 4], "b2t")
        b3t = fixed([128, 2], "b3t")
        b4p = fixed([3, 1], "b4p")
        obsA = fixed([128, N], "obsA", BF16)
        At = fixed([128, N], "At", BF16)
        Ct = fixed([128, N], "Ct", BF16)
        obsA2 = fixed([32, N], "obsA2", BF16)
        At2 = fixed([32, N], "At2", BF16)
        Ct2 = fixed([32, N], "Ct2", BF16)

        def bandof(s):
            return (obsA, At, Ct, 32 * s) if s < 3 else (obsA2, At2, Ct2, 0)
        Jc_ = [fixed([128, N], f"Jc{i}", BF16) for i in range(2)]
        R_ = [fixed([128, N], f"Rreg{i}", BF16) for i in range(2)]
        est_ = [fixed([3, N], f"est{i}", BF16) for i in range(2)]
        es2_ = [fixed([3, N], f"es2{i}", BF16) for i in range(2)]
        SES = fixed([3, BS], "SES")
        M1 = fixed([128, NT * BS], "M1")
        M2 = fixed([128, NT * BS], "M2")
        Ft = fixed([128, 8], "Ft")
        dumpx = fixed([128, N], "dumpx", BF16)
        ones_c = fixed([128, 1], "ones_c")
        outs = fixed([1, 8], "outs")

        h1A = [fixed([128, 4, N], f"h1A{i}", FP8) for i in range(BS)]
        h2p = ctx.enter_context(tc.tile_pool(name="h2", bufs=2))
        h3p = ctx.enter_context(tc.tile_pool(name="h3", bufs=2))
        psA = ctx.enter_context(tc.tile_pool(name="psA", bufs=2, space="PSUM"))
        psG = ctx.enter_context(tc.tile_pool(name="psG", bufs=2, space="PSUM"))

        # ---------- startup ----------
        # spread triggers across queues: each dma_start costs ~0.6us of its
        # issuing sequencer; sample-0 critical path (w1o4/latb/obsA) goes first
        nc.scalar.dma_start(out=w1o4, in_=W1od)
        nc.scalar.dma_start(out=latb, in_=latb_d)
        nc.scalar.dma_start(out=obsA2, in_=obs2_d)
        nc.scalar.dma_start(out=b2t, in_=b2d)
        nc.sync.dma_start(out=obsA, in_=obs_d)
        nc.sync.dma_start(out=At, in_=ainit_d)
        nc.sync.dma_start(out=Ct, in_=cinit_d)
        nc.sync.dma_start(out=At2, in_=ainit2_d)
        nc.sync.dma_start(out=Ct2, in_=cinit2_d)
        nc.sync.dma_start(out=eye34, in_=eye3d)
        nc.sync.dma_start(out=eye128, in_=eye128d)
        nc.gpsimd.dma_start(out=w2t, in_=W2d)
        nc.gpsimd.dma_start(out=w3t, in_=W3d)
        nc.gpsimd.dma_start(out=w4t, in_=W4d)
        nc.gpsimd.dma_start(out=b3t, in_=b3d)
        nc.gpsimd.dma_start(out=b4p, in_=b4d)
        nc.vector.memset(Ft, 0.0)
        nc.vector.memset(ones_c, 1.0)

        # ---------- per-sample gram rounds (generator; interleaved with next MLP) ----------
        def gram_rounds(s):
            # tiles drained by an ACT copy (then DVE runs 2x out of SBUF);
            # last sample: everything on ACT/SBUF -- nothing overlaps the tail
            act_tiles = (0, 3, 6) if s < BS - 1 else tuple(range(NT))
            R = R_[s % 2]
            _, Ats, Cts, base = bandof(s)
            nact = 0
            t0src = None
            for t in range(NT):
                gp = psG.tile([128, 1024], F32, tag="g", name=f"gp{s}_{t}")
                for j in range(2):
                    nc.tensor.matmul(
                        gp[:, 512 * j:512 * (j + 1)],
                        Ats[base:base + 7, 128 * t:128 * (t + 1)],
                        Cts[base:base + 7, 512 * j:512 * (j + 1)],
                        start=True, stop=True)
                col = M1[:, NT * s + t:NT * s + t + 1]
                if t in act_tiles:
                    jc = Jc_[nact % 2]
                    nact += 1
                    nc.scalar.activation(jc, gp[:, :], ACTF.Copy)
                    src_ = jc
                else:
                    src_ = gp[:, :]
                nc.vector.tensor_reduce(out=col, in_=src_, axis=AX.X, op=OP.max)
                if t == 0:
                    t0src = src_  # defer R init: first TT combines t0 and t1
                elif t == 1:
                    nc.vector.tensor_tensor(out=R, in0=src_, in1=t0src, op=OP.max)
                else:
                    nc.vector.tensor_tensor(out=R, in0=src_, in1=R, op=OP.max)
                yield
            # dir2 finish: transpose running colmax R, then per-block row-max
            rt = psG.tile([128, NT, 128], BF16, tag="g", name=f"rt{s}")
            for k in range(NT):
                nc.tensor.transpose(rt[:, k, :], R[:, 128 * k:128 * (k + 1)],
                                    eye128)
            yield
            nc.vector.tensor_reduce(out=M2[:, NT * s:NT * (s + 1)],
                                    in_=rt[:, :, :], axis=AX.X, op=OP.max)
            yield

        def l1_rounds(s):
            obsAs, _, _, base = bandof(s)
            obsT = obsAs[base:base + 3, :]
            h1t = h1A[s]
            for c in range(4):
                ps = psA.tile([128, N], F32, tag="a", name=f"l1ps{s}_{c}")
                for j in range(2):
                    nc.tensor.matmul(ps[:, 512 * j:512 * (j + 1)],
                                     w1o4[base:base + 3, 128 * c:128 * (c + 1)],
                                     obsT[:, 512 * j:512 * (j + 1)],
                                     start=True, stop=True)
                nc.scalar.activation(h1t[:, c, :], ps[:, :], ACTF.Relu,
                                     bias=latb[:, c, s:s + 1])
                yield

        def weave(gens, pattern):
            """round-robin generators by index pattern, one sub-step per yield"""
            live = [iter(g) if g is not None else None for g in gens]
            while any(v is not None for v in live):
                for i in pattern:
                    if live[i] is not None:
                        try:
                            next(live[i])
                        except StopIteration:
                            live[i] = None
                            continue
                        yield

        def advance(it):
            if it is not None:
                next(it, None)

        # ---------- per-sample MLP ----------
        def mlp(s, hooks):
            obsAs, Ats, Cts, base = bandof(s)
            obsT = obsAs[base:base + 3, :]
            h1t = h1A[s]
            h2t = h2p.tile([128, 4, N], FP8, tag="h2", name=f"h2_{s}")
            for c in range(4):
                ps = psA.tile([128, N], F32, tag="a", name=f"l2ps{s}_{c}")
                for j in range(2):
                    for k in range(2):
                        nc.tensor.matmul(ps[:, 512 * j:512 * (j + 1)],
                                         w2t[:, 2 * k:2 * k + 2, 128 * c:128 * (c + 1)],
                                         h1t[:, 2 * k:2 * k + 2, 512 * j:512 * (j + 1)],
                                         start=(k == 0), stop=(k == 1),
                                         perf_mode=mybir.MatmulPerfMode.DoubleRow)
                nc.scalar.activation(h2t[:, c, :], ps[:, :], ACTF.Relu,
                                     bias=b2t[:, c:c + 1])
                advance(hooks)
                advance(hooks)

            h3t = h3p.tile([128, 2, N], BF16, tag="h3", name=f"h3_{s}")
            for c in range(2):
                ps = psA.tile([128, N], F32, tag="a", name=f"l3ps{s}_{c}")
                for j in range(2):
                    for k in range(2):
                        nc.tensor.matmul(ps[:, 512 * j:512 * (j + 1)],
                                         w3t[:, 2 * k:2 * k + 2, 128 * c:128 * (c + 1)],
                                         h2t[:, 2 * k:2 * k + 2, 512 * j:512 * (j + 1)],
                                         start=(k == 0), stop=(k == 1),
                                         perf_mode=mybir.MatmulPerfMode.DoubleRow)
                nc.scalar.activation(h3t[:, c, :], ps[:, :], ACTF.Relu,
                                     bias=b3t[:, c:c + 1])
                advance(hooks)
                advance(hooks)

            ps4 = psA.tile([128, N], F32, tag="a", name=f"l4ps{s}")
            for j in range(2):
                for k in range(2):
                    nc.tensor.matmul(ps4[0:3, 512 * j:512 * (j + 1)],
                                     w4t[:, k, :],
                                     h3t[:, k, 512 * j:512 * (j + 1)],
                                     start=(k == 0), stop=False)
                # obs folded into the same accumulation via identity rows
                nc.tensor.matmul(ps4[0:3, 512 * j:512 * (j + 1)],
                                 eye34[base:base + 3, :],
                                 obsT[:, 512 * j:512 * (j + 1)],
                                 start=False, stop=True)
            est, es2 = est_[s % 2], es2_[s % 2]
            nc.scalar.activation(est[:, :], ps4[0:3, :],
                                 ACTF.Identity, bias=b4p[:, 0:1])
            nc.scalar.activation(es2[:, :], est[:, :], ACTF.Square,
                                 accum_out=SES[:, s:s + 1])
            # move est / est^2 into the gram rhs band for this sample
            nc.gpsimd.dma_start(out=Cts[base:base + 3, :], in_=est[:, :])
            nc.gpsimd.dma_start(out=Cts[base + 3:base + 6, :], in_=es2[:, :])
            # cross term sum(gt*est) for the L2 loss
            crosscol = 4 if s < 3 else 5
            nc.vector.scalar_tensor_tensor(
                out=dumpx[base:base + 3, :], in0=Ats[base:base + 3, :],
                scalar=0.0, in1=Cts[base:base + 3, :], op0=OP.add, op1=OP.mult,
                accum_out=Ft[base:base + 3, crosscol:crosscol + 1])
            advance(hooks)

        for _ in l1_rounds(0):
            pass
        pending = weave([l1_rounds(1)], [0])
        for s in range(BS):
            mlp(s, pending)
            if pending is not None:
                for _ in pending:
                    pass
            def _nops(n):
                for _ in range(n):
                    yield

            nxt = [gram_rounds(s)]
            pat = [0, 0, 1]
            if s + 2 < BS:
                nxt.append(l1_rounds(s + 2))
            else:
                nxt.append(_nops(6))
                pat = [0, 1]
            pending = weave(nxt, pat)
        if pending is not None:
            for _ in pending:
                pass

        # ---------- finale ----------
        nc.vector.tensor_reduce(out=Ft[:, 0:1], in_=M1[:, :], axis=AX.X, op=OP.add)
        nc.vector.tensor_reduce(out=Ft[:, 1:2], in_=M2[:, :], axis=AX.X, op=OP.add)
        nc.vector.tensor_reduce(out=Ft[0:3, 3:4], in_=SES[:, :], axis=AX.X, op=OP.add)

        fps = psG.tile([128, 1024], F32, tag="g", name="fps")
        nc.tensor.matmul(fps[0:1, 0:8], ones_c[:, :], Ft[:, :],
                         start=True, stop=True)
        nc.scalar.activation(outs[:, :], fps[0:1, 0:8], ACTF.Copy)
        nc.sync.dma_start(out=out_d, in_=outs)
        if DEBUG_DUMPS:
            nc.sync.dma_start(out=m1_d, in_=M1)
            nc.sync.dma_start(out=m2_d, in_=M2)
            nc.sync.dma_start(out=r0_d, in_=R_[0])
            nc.sync.dma_start(out=r1_d, in_=R_[1])

    nc.compile()
    return nc


_program_cache = []


def kernel(**inputs):
    global LAST
    if not _program_cache:
        _program_cache.append(build_program())
    nc = _program_cache[0]

    def f32(x):
        return np.ascontiguousarray(np.asarray(x, dtype=np.float32))

    W1 = np.asarray(inputs["W1"], np.float32)
    W2 = np.asarray(inputs["W2"], np.float32)
    W3 = np.asarray(inputs["W3"], np.float32)
    W4 = np.asarray(inputs["W4"], np.float32)
    b1 = np.asarray(inputs["b1"], np.float32)
    latent = np.asarray(inputs["latent"], np.float32)
    obs = np.asarray(inputs["obs"], np.float32)
    gt = np.asarray(inputs["obs_gt"], np.float32)

    w1o4 = np.zeros((128, 512), np.float32)
    eye34 = np.zeros((128, 3), np.float32)
    for s in range(3):
        w1o4[32 * s:32 * s + 3] = W1[0:3, :]
        eye34[32 * s:32 * s + 3] = np.eye(3, dtype=np.float32)

    shared = {
        "w1o4": np.ascontiguousarray(w1o4.astype(ml_dtypes.bfloat16)),
        "eye34": np.ascontiguousarray(eye34.astype(ml_dtypes.bfloat16)),
        "eye128": np.eye(128, dtype=ml_dtypes.bfloat16),
        "w2p": np.ascontiguousarray(W2.reshape(4, 128, 512).transpose(1, 0, 2).astype(ml_dtypes.float8_e4m3)),
        "b2p": f32(np.asarray(inputs["b2"], np.float32).reshape(4, 128).T),
        "w3p": np.ascontiguousarray(W3.reshape(4, 128, 256).transpose(1, 0, 2).astype(ml_dtypes.float8_e4m3)),
        "b3p": f32(np.asarray(inputs["b3"], np.float32).reshape(2, 128).T),
        "w4p": np.ascontiguousarray(W4.reshape(2, 128, 3).transpose(1, 0, 2).astype(ml_dtypes.bfloat16)),
        "b4p": f32(np.asarray(inputs["b4"], np.float32).reshape(3, 1)),
    }
    # latent bias: per-sample vector, tiny -> precompute on host
    lb_all = latent @ W1[3:, :] + b1  # [B, 512]

    in_maps = []
    for c in range(NCORES):
        sl = slice(c * BS, (c + 1) * BS)
        m = dict(shared)
        m["latb_t"] = f32(lb_all[sl].reshape(BS, 4, 128).transpose(2, 1, 0))
        obsc = obs[sl]                    # [BS, N, 3]
        gtc = gt[sl]                      # [BS, N, 3]
        g2 = (gtc * gtc).sum(-1)          # [BS, N]
        O = np.zeros((160, N), np.float32)
        A = np.zeros((160, N), np.float32)
        C = np.zeros((160, N), np.float32)
        for s in range(BS):
            r = 32 * s if s < 3 else 128
            O[r:r + 3] = obsc[s].T
            A[r:r + 3] = gtc[s].T
            A[r + 3:r + 6] = -0.5
            A[r + 6] = -0.5 * g2[s]
            C[r + 6] = 1.0
        m["obs_t"] = np.ascontiguousarray(O[:128].astype(ml_dtypes.bfloat16))
        m["a_init"] = np.ascontiguousarray(A[:128].astype(ml_dtypes.bfloat16))
        m["c_init"] = np.ascontiguousarray(C[:128].astype(ml_dtypes.bfloat16))
        m["obs_t2"] = np.ascontiguousarray(O[128:].astype(ml_dtypes.bfloat16))
        m["a_init2"] = np.ascontiguousarray(A[128:].astype(ml_dtypes.bfloat16))
        m["c_init2"] = np.ascontiguousarray(C[128:].astype(ml_dtypes.bfloat16))
        in_maps.append(m)

    res = run_bass_kernel_spmd(nc, in_maps, core_ids=list(range(NCORES)),
                               trace=TRACE)
    LAST = res

    parts = np.stack([r["partials"][0] for r in res.results]).astype(np.float64)
    s_maxJ1 = parts[:, 0].sum()
    s_maxJ2 = parts[:, 1].sum()
    s_est2 = parts[:, 3].sum()
    s_cross = parts[:, 4].sum() + parts[:, 5].sum()
    s_gt2 = float((gt.astype(np.float64) ** 2).sum())
    chm = (-2.0 * s_maxJ1 - 2.0 * s_maxJ2) / (B * N)
    l2 = (s_gt2 - 2.0 * s_cross + s_est2) / (B * N * 3)
    loss = 0.2 * chm + 0.8 * l2
    return np.asarray(loss, dtype=np.float32)


# revision 15
# speedup vs baseline: 1.2505x; 1.2505x over previous
"""Trainium2 Bass kernel for nn_DeepLatent loss (chamfer + L2 of a per-point MLP).

Strategy (8 cores, data-parallel over batch B=32 -> 4 samples/core):
  Per core, per sample s (channel-major layout: activations stored [C, Npoints]):
    h1 = relu(W1o.T @ obs^T + latbias)        latbias precomputed on host
    h2 = relu(W2.T @ h1 + b2)
    h3 = relu(W3.T @ h2 + b3)
    delta = W4.T @ h3                         est = obs + delta + b4
  Chamfer via a single augmented gram J = -d^2/2 (K=7 matmul):
    J[n,m] = gt_n . est_m - |est_m|^2/2 - |gt_n|^2/2
    lhsT rows: gt coords (3) | -0.5 (3) | -0.5*|gt|^2 (1, host-precomputed)
    rhs  rows: est coords (3) | est^2 (3) | ones (1)
    dir1 (per gt):  fused DVE tensor_tensor_reduce: J tile -> bf16 SBUF copy
                    + row-max accumulator in one pass.
    dir2 (per est): running elementwise bf16 max across the 8 gt tiles, then
                    8 PE transposes + one 3D-AP max-reduce.
  Sample s occupies partition band 32s (PE small-operand bases must be
  0/32/64/96 and lhsT/rhs must share a base).
  Scalar partials (max-sums, est^2 sum, cross sum) are combined on the host;
  sum(gt^2) is a pure input function computed on host.
"""

import ml_dtypes
import numpy as np
from contextlib import ExitStack

import concourse.bass as bass
import concourse.bacc as bacc
import concourse.mybir as mybir
import concourse.tile as tile
from concourse.bass_utils import run_bass_kernel_spmd

F32 = mybir.dt.float32
BF16 = mybir.dt.bfloat16
FP8 = mybir.dt.float8e4
AX = mybir.AxisListType
OP = mybir.AluOpType
ACTF = mybir.ActivationFunctionType

B, N, L = 32, 1024, 256
NCORES = 8
BS = B // NCORES  # samples per core
NT = N // 128     # gram tiles per sample
NEG = -3.0e38

# test.py hooks
TRACE = False
LAST = None
DEBUG_DUMPS = False


def build_program():
    nc = bacc.Bacc()

    obs_d = nc.dram_tensor("obs_t", [128, N], BF16, kind="ExternalInput")[:]
    ainit_d = nc.dram_tensor("a_init", [128, N], BF16, kind="ExternalInput")[:]
    cinit_d = nc.dram_tensor("c_init", [128, N], BF16, kind="ExternalInput")[:]
    obs2_d = nc.dram_tensor("obs_t2", [32, N], BF16, kind="ExternalInput")[:]
    ainit2_d = nc.dram_tensor("a_init2", [32, N], BF16, kind="ExternalInput")[:]
    cinit2_d = nc.dram_tensor("c_init2", [32, N], BF16, kind="ExternalInput")[:]
    latb_d = nc.dram_tensor("latb_t", [128, 4, BS], F32, kind="ExternalInput")[:]
    W1od = nc.dram_tensor("w1o4", [128, 512], BF16, kind="ExternalInput")[:]
    eye3d = nc.dram_tensor("eye34", [128, 3], BF16, kind="ExternalInput")[:]
    eye128d = nc.dram_tensor("eye128", [128, 128], BF16, kind="ExternalInput")[:]
    W2d = nc.dram_tensor("w2p", [128, 4, 512], BF16, kind="ExternalInput")[:]
    b2d = nc.dram_tensor("b2p", [128, 4], F32, kind="ExternalInput")[:]
    W3d = nc.dram_tensor("w3p", [128, 4, 256], BF16, kind="ExternalInput")[:]
    b3d = nc.dram_tensor("b3p", [128, 2], F32, kind="ExternalInput")[:]
    W4d = nc.dram_tensor("w4p", [128, 2, 3], BF16, kind="ExternalInput")[:]
    b4d = nc.dram_tensor("b4p", [3, 1], F32, kind="ExternalInput")[:]
    out_d = nc.dram_tensor("partials", [1, 8], F32, kind="ExternalOutput")[:]
    if DEBUG_DUMPS:
        m1_d = nc.dram_tensor("dbg_m1", [128, NT * BS], F32, kind="ExternalOutput")[:]
        m2_d = nc.dram_tensor("dbg_m2", [128, NT * BS], F32, kind="ExternalOutput")[:]
        r0_d = nc.dram_tensor("dbg_r0", [128, N], BF16, kind="ExternalOutput")[:]
        r1_d = nc.dram_tensor("dbg_r1", [128, N], BF16, kind="ExternalOutput")[:]

    with tile.TileContext(nc) as tc, ExitStack() as ctx:
        singles = ctx.enter_context(tc.tile_pool(name="singles", bufs=1))

        def fixed(shape, name, dtype=F32):
            return singles.tile(shape, dtype, tag=name, name=name)

        # ---------- fixed tiles ----------
        w1o4 = fixed([128, 512], "w1o4", BF16)
        eye34 = fixed([128, 3], "eye34", BF16)
        eye128 = fixed([128, 128], "eye128", BF16)
        latb = fixed([128, 4, BS], "latb")
        w2t = fixed([128, 4, 512], "w2t", BF16)
        w3t = fixed([128, 4, 256], "w3t", BF16)
        w4t = fixed([128, 2, 3], "w4t", BF16)
        b2t = fixed([128, 4], "b2t")
        b3t = fixed([128, 2], "b3t")
        b4p = fixed([3, 1], "b4p")
        obsA = fixed([128, N], "obsA", BF16)
        At = fixed([128, N], "At", BF16)
        Ct = fixed([128, N], "Ct", BF16)
        obsA2 = fixed([32, N], "obsA2", BF16)
        At2 = fixed([32, N], "At2", BF16)
        Ct2 = fixed([32, N], "Ct2", BF16)

        def bandof(s):
            return (obsA, At, Ct, 32 * s) if s < 3 else (obsA2, At2, Ct2, 0)
        Jc_ = [fixed([128, N], f"Jc{i}", BF16) for i in range(2)]
        R_ = [fixed([128, N], f"Rreg{i}", BF16) for i in range(2)]
        est_ = [fixed([3, N], f"est{i}", BF16) for i in range(2)]
        es2_ = [fixed([3, N], f"es2{i}", BF16) for i in range(2)]
        SES = fixed([3, BS], "SES")
        M1 = fixed([128, NT * BS], "M1")
        M2 = fixed([128, NT * BS], "M2")
        Ft = fixed([128, 8], "Ft")
        dumpx = fixed([128, N], "dumpx", BF16)
        ones_c = fixed([128, 1], "ones_c")
        outs = fixed([1, 8], "outs")

        h1A = [fixed([128, 4, N], f"h1A{i}", BF16) for i in range(BS)]
        h2p = ctx.enter_context(tc.tile_pool(name="h2", bufs=2))
        h3p = ctx.enter_context(tc.tile_pool(name="h3", bufs=2))
        psA = ctx.enter_context(tc.tile_pool(name="psA", bufs=2, space="PSUM"))
        psG = ctx.enter_context(tc.tile_pool(name="psG", bufs=2, space="PSUM"))

        # ---------- startup ----------
        # spread triggers across queues: each dma_start costs ~0.6us of its
        # issuing sequencer; sample-0 critical path (w1o4/latb/obsA) goes first
        nc.scalar.dma_start(out=w1o4, in_=W1od)
        nc.scalar.dma_start(out=latb, in_=latb_d)
        for q in range(4):
            nc.sync.dma_start(out=obsA[32 * q:32 * (q + 1), :],
                              in_=obs_d[32 * q:32 * (q + 1), :])
        for q in range(4):
            nc.gpsimd.dma_start(out=w2t[:, q, :], in_=W2d[:, q, :])
        nc.scalar.dma_start(out=obsA2, in_=obs2_d)
        nc.scalar.dma_start(out=b2t, in_=b2d)
        for q in range(2):
            nc.gpsimd.dma_start(out=w3t[:, 2 * q:2 * q + 2, :],
                                in_=W3d[:, 2 * q:2 * q + 2, :])
        for q in range(2):
            nc.sync.dma_start(out=At[64 * q:64 * (q + 1), :],
                              in_=ainit_d[64 * q:64 * (q + 1), :])
            nc.sync.dma_start(out=Ct[64 * q:64 * (q + 1), :],
                              in_=cinit_d[64 * q:64 * (q + 1), :])
        nc.sync.dma_start(out=At2, in_=ainit2_d)
        nc.sync.dma_start(out=Ct2, in_=cinit2_d)
        nc.scalar.dma_start(out=eye34, in_=eye3d)
        nc.scalar.dma_start(out=eye128, in_=eye128d)
        nc.gpsimd.dma_start(out=w4t, in_=W4d)
        nc.gpsimd.dma_start(out=b3t, in_=b3d)
        nc.gpsimd.dma_start(out=b4p, in_=b4d)
        nc.vector.memset(Ft, 0.0)
        nc.vector.memset(ones_c, 1.0)

        # ---------- per-sample gram rounds (generator; interleaved with next MLP) ----------
        def gram_rounds(s):
            # tiles drained by an ACT copy (then DVE runs 2x out of SBUF);
            # last sample: everything on ACT/SBUF -- nothing overlaps the tail
            act_tiles = (0, 3, 6) if s < BS - 1 else tuple(range(NT))
            R = R_[s % 2]
            _, Ats, Cts, base = bandof(s)
            nact = 0
            t0src = None
            for t in range(NT):
                gp = psG.tile([128, 1024], F32, tag="g", name=f"gp{s}_{t}")
                for j in range(2):
                    nc.tensor.matmul(
                        gp[:, 512 * j:512 * (j + 1)],
                        Ats[base:base + 7, 128 * t:128 * (t + 1)],
                        Cts[base:base + 7, 512 * j:512 * (j + 1)],
                        start=True, stop=True)
                col = M1[:, NT * s + t:NT * s + t + 1]
                if t in act_tiles:
                    jc = Jc_[nact % 2]
                    nact += 1
                    nc.scalar.activation(jc, gp[:, :], ACTF.Copy)
                    src_ = jc
                else:
                    src_ = gp[:, :]
                nc.vector.tensor_reduce(out=col, in_=src_, axis=AX.X, op=OP.max)
                if t == 0:
                    t0src = src_  # defer R init: first TT combines t0 and t1
                elif t == 1:
                    nc.vector.tensor_tensor(out=R, in0=src_, in1=t0src, op=OP.max)
                else:
                    nc.vector.tensor_tensor(out=R, in0=src_, in1=R, op=OP.max)
                yield
            # dir2 finish: transpose running colmax R, then per-block row-max
            rt = psG.tile([128, NT, 128], BF16, tag="g", name=f"rt{s}")
            for k in range(NT):
                nc.tensor.transpose(rt[:, k, :], R[:, 128 * k:128 * (k + 1)],
                                    eye128)
            yield
            nc.vector.tensor_reduce(out=M2[:, NT * s:NT * (s + 1)],
                                    in_=rt[:, :, :], axis=AX.X, op=OP.max)
            yield

        def l1_rounds(s):
            obsAs, _, _, base = bandof(s)
            obsT = obsAs[base:base + 3, :]
            h1t = h1A[s]
            for c in range(4):
                ps = psA.tile([128, N], F32, tag="a", name=f"l1ps{s}_{c}")
                for j in range(2):
                    nc.tensor.matmul(ps[:, 512 * j:512 * (j + 1)],
                                     w1o4[base:base + 3, 128 * c:128 * (c + 1)],
                                     obsT[:, 512 * j:512 * (j + 1)],
                                     start=True, stop=True)
                nc.scalar.activation(h1t[:, c, :], ps[:, :], ACTF.Relu,
                                     bias=latb[:, c, s:s + 1])
                yield

        def weave(gens, pattern):
            """round-robin generators by index pattern, one sub-step per yield"""
            live = [iter(g) if g is not None else None for g in gens]
            while any(v is not None for v in live):
                for i in pattern:
                    if live[i] is not None:
                        try:
                            next(live[i])
                        except StopIteration:
                            live[i] = None
                            continue
                        yield

        def advance(it):
            if it is not None:
                next(it, None)

        # ---------- per-sample MLP ----------
        def mlp(s, hooks):
            obsAs, Ats, Cts, base = bandof(s)
            obsT = obsAs[base:base + 3, :]
            h1t = h1A[s]
            h2t = h2p.tile([128, 4, N], BF16, tag="h2", name=f"h2_{s}")
            for c in range(4):
                ps = psA.tile([128, N], F32, tag="a", name=f"l2ps{s}_{c}")
                for j in range(2):
                    for k in range(4):
                        nc.tensor.matmul(ps[:, 512 * j:512 * (j + 1)],
                                         w2t[:, k, 128 * c:128 * (c + 1)],
                                         h1t[:, k, 512 * j:512 * (j + 1)],
                                         start=(k == 0), stop=(k == 3))
                nc.scalar.activation(h2t[:, c, :], ps[:, :], ACTF.Relu,
                                     bias=b2t[:, c:c + 1])
                advance(hooks)
                advance(hooks)

            h3t = h3p.tile([128, 2, N], BF16, tag="h3", name=f"h3_{s}")
            for c in range(2):
                ps = psA.tile([128, N], F32, tag="a", name=f"l3ps{s}_{c}")
                for j in range(2):
                    for k in range(4):
                        nc.tensor.matmul(ps[:, 512 * j:512 * (j + 1)],
                                         w3t[:, k, 128 * c:128 * (c + 1)],
                                         h2t[:, k, 512 * j:512 * (j + 1)],
                                         start=(k == 0), stop=(k == 3))
                nc.scalar.activation(h3t[:, c, :], ps[:, :], ACTF.Relu,
                                     bias=b3t[:, c:c + 1])
                advance(hooks)
                advance(hooks)

            ps4 = psA.tile([128, N], F32, tag="a", name=f"l4ps{s}")
            for j in range(2):
                for k in range(2):
                    nc.tensor.matmul(ps4[0:3, 512 * j:512 * (j + 1)],
                                     w4t[:, k, :],
                                     h3t[:, k, 512 * j:512 * (j + 1)],
                                     start=(k == 0), stop=False)
                # obs folded into the same accumulation via identity rows
                nc.tensor.matmul(ps4[0:3, 512 * j:512 * (j + 1)],
                                 eye34[base:base + 3, :],
                                 obsT[:, 512 * j:512 * (j + 1)],
                                 start=False, stop=True)
            est, es2 = est_[s % 2], es2_[s % 2]
            nc.scalar.activation(est[:, :], ps4[0:3, :],
                                 ACTF.Identity, bias=b4p[:, 0:1])
            nc.scalar.activation(es2[:, :], est[:, :], ACTF.Square,
                                 accum_out=SES[:, s:s + 1])
            # move est / est^2 into the gram rhs band for this sample
            nc.gpsimd.dma_start(out=Cts[base:base + 3, :], in_=est[:, :])
            nc.gpsimd.dma_start(out=Cts[base + 3:base + 6, :], in_=es2[:, :])
            # cross term sum(gt*est) for the L2 loss
            crosscol = 4 if s < 3 else 5
            nc.vector.scalar_tensor_tensor(
                out=dumpx[base:base + 3, :], in0=Ats[base:base + 3, :],
                scalar=0.0, in1=Cts[base:base + 3, :], op0=OP.add, op1=OP.mult,
                accum_out=Ft[base:base + 3, crosscol:crosscol + 1])
            advance(hooks)

        for _ in l1_rounds(0):
            pass
        pending = weave([l1_rounds(1)], [0])
        for s in range(BS):
            mlp(s, pending)
            if pending is not None:
                for _ in pending:
                    pass
            def _nops(n):
                for _ in range(n):
                    yield

            nxt = [gram_rounds(s)]
            pat = [0, 0, 1]
            if s + 2 < BS:
                nxt.append(l1_rounds(s + 2))
            else:
                nxt.append(_nops(6))
                pat = [0, 1]
            pending = weave(nxt, pat)
        if pending is not None:
            for _ in pending:
                pass

        # ---------- finale ----------
        nc.vector.tensor_reduce(out=Ft[:, 0:1], in_=M1[:, :], axis=AX.X, op=OP.add)
        nc.vector.tensor_reduce(out=Ft[:, 1:2], in_=M2[:, :], axis=AX.X, op=OP.add)
        nc.vector.tensor_reduce(out=Ft[0:3, 3:4], in_=SES[:, :], axis=AX.X, op=OP.add)

        fps = psG.tile([128, 1024], F32, tag="g", name="fps")
        nc.tensor.matmul(fps[0:1, 0:8], ones_c[:, :], Ft[:, :],
                         start=True, stop=True)
        nc.scalar.activation(outs[:, :], fps[0:1, 0:8], ACTF.Copy)
        nc.sync.dma_start(out=out_d, in_=outs)
        if DEBUG_DUMPS:
            nc.sync.dma_start(out=m1_d, in_=M1)
            nc.sync.dma_start(out=m2_d, in_=M2)
            nc.sync.dma_start(out=r0_d, in_=R_[0])
            nc.sync.dma_start(out=r1_d, in_=R_[1])

    nc.compile()
    return nc


_program_cache = []


def kernel(**inputs):
    global LAST
    if not _program_cache:
        _program_cache.append(build_program())
    nc = _program_cache[0]

    def f32(x):
        return np.ascontiguousarray(np.asarray(x, dtype=np.float32))

    W1 = np.asarray(inputs["W1"], np.float32)
    W2 = np.asarray(inputs["W2"], np.float32)
    W3 = np.asarray(inputs["W3"], np.float32)
    W4 = np.asarray(inputs["W4"], np.float32)
    b1 = np.asarray(inputs["b1"], np.float32)
    latent = np.asarray(inputs["latent"], np.float32)
    obs = np.asarray(inputs["obs"], np.float32)
    gt = np.asarray(inputs["obs_gt"], np.float32)

    w1o4 = np.zeros((128, 512), np.float32)
    eye34 = np.zeros((128, 3), np.float32)
    for s in range(3):
        w1o4[32 * s:32 * s + 3] = W1[0:3, :]
        eye34[32 * s:32 * s + 3] = np.eye(3, dtype=np.float32)

    shared = {
        "w1o4": np.ascontiguousarray(w1o4.astype(ml_dtypes.bfloat16)),
        "eye34": np.ascontiguousarray(eye34.astype(ml_dtypes.bfloat16)),
        "eye128": np.eye(128, dtype=ml_dtypes.bfloat16),
        "w2p": np.ascontiguousarray(W2.reshape(4, 128, 512).transpose(1, 0, 2).astype(ml_dtypes.bfloat16)),
        "b2p": f32(np.asarray(inputs["b2"], np.float32).reshape(4, 128).T),
        "w3p": np.ascontiguousarray(W3.reshape(4, 128, 256).transpose(1, 0, 2).astype(ml_dtypes.bfloat16)),
        "b3p": f32(np.asarray(inputs["b3"], np.float32).reshape(2, 128).T),
        "w4p": np.ascontiguousarray(W4.reshape(2, 128, 3).transpose(1, 0, 2).astype(ml_dtypes.bfloat16)),
        "b4p": f32(np.asarray(inputs["b4"], np.float32).reshape(3, 1)),
    }
    # latent bias: per-sample vector, tiny -> precompute on host
    lb_all = latent @ W1[3:, :] + b1  # [B, 512]

    in_maps = []
    for c in range(NCORES):
        sl = slice(c * BS, (c + 1) * BS)
        m = dict(shared)
        m["latb_t"] = f32(lb_all[sl].reshape(BS, 4, 128).transpose(2, 1, 0))
        obsc = obs[sl]                    # [BS, N, 3]
        gtc = gt[sl]                      # [BS, N, 3]
        g2 = (gtc * gtc).sum(-1)          # [BS, N]
        O = np.zeros((160, N), np.float32)
        A = np.zeros((160, N), np.float32)
        C = np.zeros((160, N), np.float32)
        for s in range(BS):
            r = 32 * s if s < 3 else 128
            O[r:r + 3] = obsc[s].T
            A[r:r + 3] = gtc[s].T
            A[r + 3:r + 6] = -0.5
            A[r + 6] = -0.5 * g2[s]
            C[r + 6] = 1.0
        m["obs_t"] = np.ascontiguousarray(O[:128].astype(ml_dtypes.bfloat16))
        m["a_init"] = np.ascontiguousarray(A[:128].astype(ml_dtypes.bfloat16))
        m["c_init"] = np.ascontiguousarray(C[:128].astype(ml_dtypes.bfloat16))
        m["obs_t2"] = np.ascontiguousarray(O[128:].astype(ml_dtypes.bfloat16))
        m["a_init2"] = np.ascontiguousarray(A[128:].astype(ml_dtypes.bfloat16))
        m["c_init2"] = np.ascontiguousarray(C[128:].astype(ml_dtypes.bfloat16))
        in_maps.append(m)

    res = run_bass_kernel_spmd(nc, in_maps, core_ids=list(range(NCORES)),
                               trace=TRACE)
    LAST = res

    parts = np.stack([r["partials"][0] for r in res.results]).astype(np.float64)
    s_maxJ1 = parts[:, 0].sum()
    s_maxJ2 = parts[:, 1].sum()
    s_est2 = parts[:, 3].sum()
    s_cross = parts[:, 4].sum() + parts[:, 5].sum()
    s_gt2 = float((gt.astype(np.float64) ** 2).sum())
    chm = (-2.0 * s_maxJ1 - 2.0 * s_maxJ2) / (B * N)
    l2 = (s_gt2 - 2.0 * s_cross + s_est2) / (B * N * 3)
    loss = 0.2 * chm + 0.8 * l2
    return np.asarray(loss, dtype=np.float32)


# revision 16
# speedup vs baseline: 1.3578x; 1.0858x over previous
"""Trainium2 Bass kernel for nn_DeepLatent loss (chamfer + L2 of a per-point MLP).

Strategy (8 cores, data-parallel over batch B=32 -> 4 samples/core):
  Per core, per sample s (channel-major layout: activations stored [C, Npoints]):
    h1 = relu(W1o.T @ obs^T + latbias)        latbias precomputed on host
    h2 = relu(W2.T @ h1 + b2)
    h3 = relu(W3.T @ h2 + b3)
    delta = W4.T @ h3                         est = obs + delta + b4
  Chamfer via a single augmented gram J = -d^2/2 (K=7 matmul):
    J[n,m] = gt_n . est_m - |est_m|^2/2 - |gt_n|^2/2
    lhsT rows: gt coords (3) | -0.5 (3) | -0.5*|gt|^2 (1, host-precomputed)
    rhs  rows: est coords (3) | est^2 (3) | ones (1)
    dir1 (per gt):  fused DVE tensor_tensor_reduce: J tile -> bf16 SBUF copy
                    + row-max accumulator in one pass.
    dir2 (per est): running elementwise bf16 max across the 8 gt tiles, then
                    8 PE transposes + one 3D-AP max-reduce.
  Sample s occupies partition band 32s (PE small-operand bases must be
  0/32/64/96 and lhsT/rhs must share a base).
  Scalar partials (max-sums, est^2 sum, cross sum) are combined on the host;
  sum(gt^2) is a pure input function computed on host.
"""

import ml_dtypes
import numpy as np
from contextlib import ExitStack

import concourse.bass as bass
import concourse.bacc as bacc
import concourse.mybir as mybir
import concourse.tile as tile
from concourse.bass_utils import run_bass_kernel_spmd

F32 = mybir.dt.float32
BF16 = mybir.dt.bfloat16
FP8 = mybir.dt.float8e4
AX = mybir.AxisListType
OP = mybir.AluOpType
ACTF = mybir.ActivationFunctionType

B, N, L = 32, 1024, 256
NCORES = 8
BS = B // NCORES  # samples per core
NT = N // 128     # gram tiles per sample
NEG = -3.0e38

# test.py hooks
TRACE = False
LAST = None
DEBUG_DUMPS = False


def build_program():
    nc = bacc.Bacc()

    obs_d = nc.dram_tensor("obs_t", [128, N], BF16, kind="ExternalInput")[:]
    ainit_d = nc.dram_tensor("a_init", [128, N], BF16, kind="ExternalInput")[:]
    cinit_d = nc.dram_tensor("c_init", [128, N], BF16, kind="ExternalInput")[:]
    obs2_d = nc.dram_tensor("obs_t2", [32, N], BF16, kind="ExternalInput")[:]
    ainit2_d = nc.dram_tensor("a_init2", [32, N], BF16, kind="ExternalInput")[:]
    cinit2_d = nc.dram_tensor("c_init2", [32, N], BF16, kind="ExternalInput")[:]
    latb_d = nc.dram_tensor("latb_t", [128, 4, BS], F32, kind="ExternalInput")[:]
    W1od = nc.dram_tensor("w1o4", [128, 512], BF16, kind="ExternalInput")[:]
    eye3d = nc.dram_tensor("eye34", [128, 3], BF16, kind="ExternalInput")[:]
    eye128d = nc.dram_tensor("eye128", [128, 128], BF16, kind="ExternalInput")[:]
    W2d = nc.dram_tensor("w2p", [128, 4, 512], BF16, kind="ExternalInput")[:]
    b2d = nc.dram_tensor("b2p", [128, 4], F32, kind="ExternalInput")[:]
    W3d = nc.dram_tensor("w3p", [128, 4, 256], BF16, kind="ExternalInput")[:]
    b3d = nc.dram_tensor("b3p", [128, 2], F32, kind="ExternalInput")[:]
    W4d = nc.dram_tensor("w4p", [128, 2, 3], BF16, kind="ExternalInput")[:]
    b4d = nc.dram_tensor("b4p", [128, 1], F32, kind="ExternalInput")[:]
    out_d = nc.dram_tensor("partials", [1, 8], F32, kind="ExternalOutput")[:]
    if DEBUG_DUMPS:
        m1_d = nc.dram_tensor("dbg_m1", [128, NT * BS], F32, kind="ExternalOutput")[:]
        m2_d = nc.dram_tensor("dbg_m2", [128, NT * BS], F32, kind="ExternalOutput")[:]
        r0_d = nc.dram_tensor("dbg_r0", [128, N], BF16, kind="ExternalOutput")[:]
        r1_d = nc.dram_tensor("dbg_r1", [128, N], BF16, kind="ExternalOutput")[:]

    with tile.TileContext(nc) as tc, ExitStack() as ctx:
        singles = ctx.enter_context(tc.tile_pool(name="singles", bufs=1))

        def fixed(shape, name, dtype=F32):
            return singles.tile(shape, dtype, tag=name, name=name)

        # ---------- fixed tiles ----------
        w1o4 = fixed([128, 512], "w1o4", BF16)
        eye34 = fixed([128, 3], "eye34", BF16)
        eye128 = fixed([128, 128], "eye128", BF16)
        latb = fixed([128, 4, BS], "latb")
        w2t = fixed([128, 4, 512], "w2t", BF16)
        w3t = fixed([128, 4, 256], "w3t", BF16)
        w4t = fixed([128, 2, 3], "w4t", BF16)
        b2t = fixed([128, 4], "b2t")
        b3t = fixed([128, 2], "b3t")
        b4p = fixed([128, 1], "b4p")
        obsA = fixed([128, N], "obsA", BF16)
        At = fixed([128, N], "At", BF16)
        Ct = fixed([128, N], "Ct", BF16)
        obsA2 = fixed([32, N], "obsA2", BF16)
        At2 = fixed([32, N], "At2", BF16)
        Ct2 = fixed([32, N], "Ct2", BF16)

        def bandof(s):
            return (obsA, At, Ct, 32 * s) if s < 3 else (obsA2, At2, Ct2, 0)
        Jc_ = [fixed([128, N], f"Jc{i}", BF16) for i in range(2)]
        R_ = [fixed([128, N], f"Rreg{i}", BF16) for i in range(2)]
        SES = fixed([128, BS], "SES")
        M1 = fixed([128, NT * BS], "M1")
        M2 = fixed([128, NT * BS], "M2")
        Ft = fixed([128, 8], "Ft")
        dumpx = fixed([128, N], "dumpx", BF16)
        ones_c = fixed([128, 1], "ones_c")
        outs = fixed([1, 8], "outs")

        h1A = [fixed([128, 4, N], f"h1A{i}", BF16) for i in range(BS)]
        h2p = ctx.enter_context(tc.tile_pool(name="h2", bufs=2))
        h3p = ctx.enter_context(tc.tile_pool(name="h3", bufs=2))
        psA = ctx.enter_context(tc.tile_pool(name="psA", bufs=2, space="PSUM"))
        psG = ctx.enter_context(tc.tile_pool(name="psG", bufs=2, space="PSUM"))

        # ---------- startup ----------
        # spread triggers across queues: each dma_start costs ~0.6us of its
        # issuing sequencer; sample-0 critical path (w1o4/latb/obsA) goes first
        nc.scalar.dma_start(out=w1o4, in_=W1od)
        nc.scalar.dma_start(out=latb, in_=latb_d)
        for q in range(4):
            nc.sync.dma_start(out=obsA[32 * q:32 * (q + 1), :],
                              in_=obs_d[32 * q:32 * (q + 1), :])
        for q in range(4):
            nc.gpsimd.dma_start(out=w2t[:, q, :], in_=W2d[:, q, :])
        nc.scalar.dma_start(out=obsA2, in_=obs2_d)
        nc.scalar.dma_start(out=b2t, in_=b2d)
        for q in range(2):
            nc.gpsimd.dma_start(out=w3t[:, 2 * q:2 * q + 2, :],
                                in_=W3d[:, 2 * q:2 * q + 2, :])
        for q in range(2):
            nc.sync.dma_start(out=At[64 * q:64 * (q + 1), :],
                              in_=ainit_d[64 * q:64 * (q + 1), :])
            nc.sync.dma_start(out=Ct[64 * q:64 * (q + 1), :],
                              in_=cinit_d[64 * q:64 * (q + 1), :])
        nc.sync.dma_start(out=At2, in_=ainit2_d)
        nc.sync.dma_start(out=Ct2, in_=cinit2_d)
        nc.scalar.dma_start(out=eye34, in_=eye3d)
        nc.scalar.dma_start(out=eye128, in_=eye128d)
        nc.gpsimd.dma_start(out=w4t, in_=W4d)
        nc.gpsimd.dma_start(out=b3t, in_=b3d)
        nc.gpsimd.dma_start(out=b4p, in_=b4d)
        nc.vector.memset(Ft, 0.0)
        nc.vector.memset(SES, 0.0)
        # trigger the ACT function-table load before real work arrives
        nc.scalar.activation(outs[:, :], Ft[0:1, 0:8], ACTF.Copy)
        nc.vector.memset(ones_c, 1.0)

        # ---------- per-sample gram rounds (generator; interleaved with next MLP) ----------
        def gram_rounds(s):
            # tiles drained by an ACT copy (then DVE runs 2x out of SBUF);
            # last sample: everything on ACT/SBUF -- nothing overlaps the tail
            act_tiles = (0, 2, 4, 6) if s < BS - 1 else tuple(range(NT))
            R = R_[s % 2]
            _, Ats, Cts, base = bandof(s)
            nact = 0
            t0src = None
            for t in range(NT):
                gp = psG.tile([128, 1024], F32, tag="g", name=f"gp{s}_{t}")
                for j in range(2):
                    nc.tensor.matmul(
                        gp[:, 512 * j:512 * (j + 1)],
                        Ats[base:base + 7, 128 * t:128 * (t + 1)],
                        Cts[base:base + 7, 512 * j:512 * (j + 1)],
                        start=True, stop=True)
                col = M1[:, NT * s + t:NT * s + t + 1]
                if t in act_tiles:
                    jc = Jc_[nact % 2]
                    nact += 1
                    nc.scalar.activation(jc, gp[:, :], ACTF.Copy)
                    src_ = jc
                else:
                    src_ = gp[:, :]
                if t == 0:
                    t0src = src_  # defer R init: first TT combines t0 and t1
                elif t == 1:
                    nc.vector.tensor_tensor(out=R, in0=src_, in1=t0src, op=OP.max)
                else:
                    nc.vector.tensor_tensor(out=R, in0=src_, in1=R, op=OP.max)
                nc.vector.tensor_reduce(out=col, in_=src_, axis=AX.X, op=OP.max)
                yield
            # dir2 finish: transpose running colmax R, then per-block row-max
            rt = psG.tile([128, NT, 128], BF16, tag="g", name=f"rt{s}")
            for k in range(NT):
                nc.tensor.transpose(rt[:, k, :], R[:, 128 * k:128 * (k + 1)],
                                    eye128)
            yield
            nc.vector.tensor_reduce(out=M2[:, NT * s:NT * (s + 1)],
                                    in_=rt[:, :, :], axis=AX.X, op=OP.max)
            yield

        def l1_rounds(s):
            obsAs, _, _, base = bandof(s)
            obsT = obsAs[base:base + 3, :]
            h1t = h1A[s]
            for c in range(4):
                ps = psA.tile([128, N], F32, tag="a", name=f"l1ps{s}_{c}")
                for j in range(2):
                    nc.tensor.matmul(ps[:, 512 * j:512 * (j + 1)],
                                     w1o4[base:base + 3, 128 * c:128 * (c + 1)],
                                     obsT[:, 512 * j:512 * (j + 1)],
                                     start=True, stop=True)
                nc.scalar.activation(h1t[:, c, :], ps[:, :], ACTF.Relu,
                                     bias=latb[:, c, s:s + 1])
                yield

        def weave(gens, pattern):
            """round-robin generators by index pattern, one sub-step per yield"""
            live = [iter(g) if g is not None else None for g in gens]
            while any(v is not None for v in live):
                for i in pattern:
                    if live[i] is not None:
                        try:
                            next(live[i])
                        except StopIteration:
                            live[i] = None
                            continue
                        yield

        def advance(it):
            if it is not None:
                next(it, None)

        # ---------- per-sample MLP ----------
        def mlp(s, hooks):
            obsAs, Ats, Cts, base = bandof(s)
            obsT = obsAs[base:base + 3, :]
            h1t = h1A[s]
            h2t = h2p.tile([128, 4, N], BF16, tag="h2", name=f"h2_{s}")
            for c in range(4):
                ps = psA.tile([128, N], F32, tag="a", name=f"l2ps{s}_{c}")
                for j in range(2):
                    for k in range(4):
                        nc.tensor.matmul(ps[:, 512 * j:512 * (j + 1)],
                                         w2t[:, k, 128 * c:128 * (c + 1)],
                                         h1t[:, k, 512 * j:512 * (j + 1)],
                                         start=(k == 0), stop=(k == 3))
                nc.scalar.activation(h2t[:, c, :], ps[:, :], ACTF.Relu,
                                     bias=b2t[:, c:c + 1])
                advance(hooks)
                advance(hooks)

            h3t = h3p.tile([128, 2, N], BF16, tag="h3", name=f"h3_{s}")
            for c in range(2):
                ps = psA.tile([128, N], F32, tag="a", name=f"l3ps{s}_{c}")
                for j in range(2):
                    for k in range(4):
                        nc.tensor.matmul(ps[:, 512 * j:512 * (j + 1)],
                                         w3t[:, k, 128 * c:128 * (c + 1)],
                                         h2t[:, k, 512 * j:512 * (j + 1)],
                                         start=(k == 0), stop=(k == 3))
                nc.scalar.activation(h3t[:, c, :], ps[:, :], ACTF.Relu,
                                     bias=b3t[:, c:c + 1])
                advance(hooks)
                advance(hooks)

            ps4 = psA.tile([128, N], F32, tag="a", name=f"l4ps{s}")
            for j in range(2):
                for k in range(2):
                    nc.tensor.matmul(ps4[base:base + 3, 512 * j:512 * (j + 1)],
                                     w4t[:, k, :],
                                     h3t[:, k, 512 * j:512 * (j + 1)],
                                     start=(k == 0), stop=False)
                # obs folded into the same accumulation via identity rows
                nc.tensor.matmul(ps4[base:base + 3, 512 * j:512 * (j + 1)],
                                 eye34[base:base + 3, :],
                                 obsT[:, 512 * j:512 * (j + 1)],
                                 start=False, stop=True)
            # est straight into the gram rhs band (same partitions as ps4)
            nc.scalar.activation(Cts[base:base + 3, :], ps4[base:base + 3, :],
                                 ACTF.Identity, bias=b4p[base:base + 3, 0:1])
            nc.scalar.activation(dumpx[base:base + 3, :], Cts[base:base + 3, :],
                                 ACTF.Square, accum_out=SES[base:base + 3, s:s + 1])
            nc.gpsimd.dma_start(out=Cts[base + 3:base + 6, :],
                                in_=dumpx[base:base + 3, :])
            # cross term sum(gt*est) for the L2 loss
            crosscol = 4 if s < 3 else 5
            nc.vector.scalar_tensor_tensor(
                out=dumpx[base:base + 3, :], in0=Ats[base:base + 3, :],
                scalar=0.0, in1=Cts[base:base + 3, :], op0=OP.add, op1=OP.mult,
                accum_out=Ft[base:base + 3, crosscol:crosscol + 1])
            advance(hooks)

        for _ in l1_rounds(0):
            pass
        pending = weave([l1_rounds(1)], [0])
        for s in range(BS):
            mlp(s, pending)
            if pending is not None:
                for _ in pending:
                    pass
            def _nops(n):
                for _ in range(n):
                    yield

            nxt = [gram_rounds(s)]
            pat = [0, 0, 1]
            if s + 2 < BS:
                nxt.append(l1_rounds(s + 2))
            else:
                nxt.append(_nops(6))
                pat = [0, 1]
            pending = weave(nxt, pat)
        if pending is not None:
            for _ in pending:
                pass

        # ---------- finale ----------
        nc.vector.tensor_reduce(out=Ft[:, 0:1], in_=M1[:, :], axis=AX.X, op=OP.add)
        nc.vector.tensor_reduce(out=Ft[:, 1:2], in_=M2[:, :], axis=AX.X, op=OP.add)
        nc.vector.tensor_reduce(out=Ft[:, 3:4], in_=SES[:, :], axis=AX.X, op=OP.add)

        fps = psG.tile([128, 1024], F32, tag="g", name="fps")
        nc.tensor.matmul(fps[0:1, 0:8], ones_c[:, :], Ft[:, :],
                         start=True, stop=True)
        nc.scalar.activation(outs[:, :], fps[0:1, 0:8], ACTF.Copy)
        nc.sync.dma_start(out=out_d, in_=outs)
        if DEBUG_DUMPS:
            nc.sync.dma_start(out=m1_d, in_=M1)
            nc.sync.dma_start(out=m2_d, in_=M2)
            nc.sync.dma_start(out=r0_d, in_=R_[0])
            nc.sync.dma_start(out=r1_d, in_=R_[1])

    nc.compile()
    return nc


_program_cache = []


_B4IDX = np.zeros(128, dtype=np.int64)
for _s in range(4):
    _b = 32 * _s if _s < 3 else 0
    for _c in range(3):
        _B4IDX[_b + _c] = 3 * _s + _c
_B4IDX = _B4IDX  # rows outside bands read b4[0]; harmless


def kernel(**inputs):
    global LAST
    if not _program_cache:
        _program_cache.append(build_program())
    nc = _program_cache[0]

    def f32(x):
        return np.ascontiguousarray(np.asarray(x, dtype=np.float32))

    W1 = np.asarray(inputs["W1"], np.float32)
    W2 = np.asarray(inputs["W2"], np.float32)
    W3 = np.asarray(inputs["W3"], np.float32)
    W4 = np.asarray(inputs["W4"], np.float32)
    b1 = np.asarray(inputs["b1"], np.float32)
    latent = np.asarray(inputs["latent"], np.float32)
    obs = np.asarray(inputs["obs"], np.float32)
    gt = np.asarray(inputs["obs_gt"], np.float32)

    w1o4 = np.zeros((128, 512), np.float32)
    eye34 = np.zeros((128, 3), np.float32)
    for s in range(3):
        w1o4[32 * s:32 * s + 3] = W1[0:3, :]
        eye34[32 * s:32 * s + 3] = np.eye(3, dtype=np.float32)

    shared = {
        "w1o4": np.ascontiguousarray(w1o4.astype(ml_dtypes.bfloat16)),
        "eye34": np.ascontiguousarray(eye34.astype(ml_dtypes.bfloat16)),
        "eye128": np.eye(128, dtype=ml_dtypes.bfloat16),
        "w2p": np.ascontiguousarray(W2.reshape(4, 128, 512).transpose(1, 0, 2).astype(ml_dtypes.bfloat16)),
        "b2p": f32(np.asarray(inputs["b2"], np.float32).reshape(4, 128).T),
        "w3p": np.ascontiguousarray(W3.reshape(4, 128, 256).transpose(1, 0, 2).astype(ml_dtypes.bfloat16)),
        "b3p": f32(np.asarray(inputs["b3"], np.float32).reshape(2, 128).T),
        "w4p": np.ascontiguousarray(W4.reshape(2, 128, 3).transpose(1, 0, 2).astype(ml_dtypes.bfloat16)),
        "b4p": f32(np.tile(np.asarray(inputs["b4"], np.float32).reshape(1, 3),
                    (4, 1)).reshape(12, 1)[_B4IDX]),
    }
    # latent bias: per-sample vector, tiny -> precompute on host
    lb_all = latent @ W1[3:, :] + b1  # [B, 512]

    in_maps = []
    for c in range(NCORES):
        sl = slice(c * BS, (c + 1) * BS)
        m = dict(shared)
        m["latb_t"] = f32(lb_all[sl].reshape(BS, 4, 128).transpose(2, 1, 0))
        obsc = obs[sl]                    # [BS, N, 3]
        gtc = gt[sl]                      # [BS, N, 3]
        g2 = (gtc * gtc).sum(-1)          # [BS, N]
        O = np.zeros((160, N), np.float32)
        A = np.zeros((160, N), np.float32)
        C = np.zeros((160, N), np.float32)
        for s in range(BS):
            r = 32 * s if s < 3 else 128
            O[r:r + 3] = obsc[s].T
            A[r:r + 3] = gtc[s].T
            A[r + 3:r + 6] = -0.5
            A[r + 6] = -0.5 * g2[s]
            C[r + 6] = 1.0
        m["obs_t"] = np.ascontiguousarray(O[:128].astype(ml_dtypes.bfloat16))
        m["a_init"] = np.ascontiguousarray(A[:128].astype(ml_dtypes.bfloat16))
        m["c_init"] = np.ascontiguousarray(C[:128].astype(ml_dtypes.bfloat16))
        m["obs_t2"] = np.ascontiguousarray(O[128:].astype(ml_dtypes.bfloat16))
        m["a_init2"] = np.ascontiguousarray(A[128:].astype(ml_dtypes.bfloat16))
        m["c_init2"] = np.ascontiguousarray(C[128:].astype(ml_dtypes.bfloat16))
        in_maps.append(m)

    res = run_bass_kernel_spmd(nc, in_maps, core_ids=list(range(NCORES)),
                               trace=TRACE)
    LAST = res

    parts = np.stack([r["partials"][0] for r in res.results]).astype(np.float64)
    s_maxJ1 = parts[:, 0].sum()
    s_maxJ2 = parts[:, 1].sum()
    s_est2 = parts[:, 3].sum()
    s_cross = parts[:, 4].sum() + parts[:, 5].sum()
    s_gt2 = float((gt.astype(np.float64) ** 2).sum())
    chm = (-2.0 * s_maxJ1 - 2.0 * s_maxJ2) / (B * N)
    l2 = (s_gt2 - 2.0 * s_cross + s_est2) / (B * N * 3)
    loss = 0.2 * chm + 0.8 * l2
    return np.asarray(loss, dtype=np.float32)
